# revision 14
# baseline (speedup 1.0000x reference)
"""GNN (2x SAGEConv + linear) Bass kernel for trn2, 8 NeuronCores.

Transposed-feature layout: h^T [64 feats (partitions), nodes (free dim)].
Gather of h[src] per edge is done with gpsimd ap_gather (all 8 Q7 cores,
SBUF source) instead of SWDGE dma_gather (2-core descriptor generation).
Per layer: 4 calls, each loading two 12544-node source blocks into SBUF
(partitions 0-63 / 64-127), gathering padded-CSR edge messages, DVE
segment-reduce along free dim into a sorted plane, ap_gather unpermute +
DVE add into acc.  Phase 2 = PE matmuls on 512-col chunks + PReLU.
One AllGather of h1^T slices between layers.  Output is out^T; host
transposes back.
"""
import numpy as np

N = 100000
E = 1250000
HID = 64
P = 8
NPC = 12500          # nodes per core (output ownership)
RPC = 12544          # padded block width (= per-core h1 slice cols)
NBLK = 8             # source blocks per layer
PADC = 8             # zero pad cols appended to window buffer
WCOLS = RPC + PADC   # 12552  (<= 32768 ap_gather limit)
PADCOL = RPC         # gather index that reads a guaranteed-zero column
CMAX = 1536          # gather chunk (idx positions per ap_gather call)
UCH = 1024           # unpermute chunk
PCH = 512            # phase2 column chunk (one PSUM bank)


def _ceil16(v):
    return -(-v // 16) * 16


def _wrap(stream):
    """[L] int16 -> [16, L/16] wrapped (idx j at row j%16, col j//16)."""
    return np.ascontiguousarray(stream.reshape(-1, 16).T)


def _plan_layer(sid, dst, blkdiv):
    """Shared tile/chunk plan + per-core gather/unpermute index streams.

    sid: effective source id per edge (block = sid//blkdiv, col = sid%blkdiv)
    dst: destination node per edge
    """
    core = (dst // NPC).astype(np.int64)
    blk = (sid // blkdiv).astype(np.int64)
    col = (sid % blkdiv).astype(np.int64)
    dstl = (dst % NPC).astype(np.int64)

    deg = np.zeros((P, NBLK, NPC), np.int32)
    np.add.at(deg, (core, blk, dstl), 1)
    degp = np.zeros((P, NBLK, RPC), np.int32)
    degp[:, :, :NPC] = deg
    order = np.argsort(-degp, axis=2, kind="stable")       # [P, NBLK, RPC]
    degs = -np.sort(-degp, axis=2)
    Dsh = degs.max(axis=0)                                  # [NBLK, RPC] shared
    fz = (Dsh > 0).sum(axis=1)                              # per block
    rank = np.argsort(order, axis=2).astype(np.int16)       # rank[c,b,node]=pos

    # edges sorted by (core, blk, dstl)
    eorder = np.lexsort((col, dstl, blk, core))
    sc, sb, sd, scol = core[eorder], blk[eorder], dstl[eorder], col[eorder]
    key = (sc * NBLK + sb) * NPC + sd
    starts = np.searchsorted(key, np.arange(P * NBLK * NPC, dtype=np.int64))
    starts = np.append(starts, len(key))
    scol16 = scol.astype(np.int16)

    # shared tile plan per block: tiles never cross CMAX chunk boundaries
    blk_plans = []
    for b in range(NBLK):
        tiles = []
        off = 0
        pos = 0
        f = int(fz[b])
        prof = Dsh[b, :f]
        while pos < f:
            D = int(prof[pos])
            room = CMAX - (off % CMAX)
            if room < D:
                off += room
                room = CMAX
            run = int(np.searchsorted(-prof[pos:], -D, side="right"))
            T = min(room // D, run)
            tiles.append((off, pos, T, D))
            off += T * D
            pos += T
        blk_plans.append((tiles, _ceil16(off)))

    calls = []
    for j in range(4):
        tA, lA = blk_plans[2 * j]
        tB, lB = blk_plans[2 * j + 1]
        L = max(lA, lB, 16)
        nch = -(-L // CMAX)
        sizes = [CMAX] * (nch - 1) + [L - CMAX * (nch - 1)]
        # per chunk, per half: tiles (local_off, pos, T, D)
        chunk_tiles = [[[], []] for _ in range(nch)]
        for h, tl in enumerate((tA, tB)):
            for (off, pos, T, D) in tl:
                k = off // CMAX
                chunk_tiles[k][h].append((off - k * CMAX, pos, T, D))
        calls.append(dict(L=L, sizes=sizes, chunk_tiles=chunk_tiles,
                          fzA=int(fz[2 * j]), fzB=int(fz[2 * j + 1])))

    Dmax = max(int(Dsh[:, 0].max()), 1)
    ar = np.arange(Dmax)
    gi_cores, ui_cores = [], []
    for c in range(P):
        gparts, uparts = [], []
        for j in range(4):
            L = calls[j]["L"]
            halves = []
            for b in (2 * j, 2 * j + 1):
                od = order[c, b]
                cnts = degp[c, b][od]
                s0 = starts[(c * NBLK + b) * NPC + np.minimum(od, NPC - 1)]
                M = np.full((RPC, Dmax), PADCOL, np.int16)
                mask = ar[None, :] < cnts[:, None]
                fi = (s0[:, None] + ar[None, :])[mask]
                M[mask] = scol16[fi]
                stream = np.full(L, PADCOL, np.int16)
                for (off, pos, T, D) in blk_plans[b][0]:
                    stream[off:off + T * D] = M[pos:pos + T, :D].ravel()
                halves.append(stream)
            wA, wB = _wrap(halves[0]), _wrap(halves[1])
            gparts.append(np.concatenate(
                [np.tile(wA, (4, 1)), np.tile(wB, (4, 1))], axis=0))
            rA, rB = _wrap(rank[c, 2 * j]), _wrap(rank[c, 2 * j + 1])
            uparts.append(np.concatenate(
                [np.tile(rA, (4, 1)), np.tile(rB, (4, 1))], axis=0))
        gi_cores.append(np.concatenate(gparts, axis=1))
        ui_cores.append(np.concatenate(uparts, axis=1))
    return calls, gi_cores, ui_cores


def kernel(x, edge_index, edge_weight, emb, Wl1, bl1, Wr1, a1,
           Wl2, bl2, Wr2, a2, Wout, bout):
    import concourse.bacc as bacc
    import concourse.mybir as mybir
    import concourse.tile as tile
    from concourse.bass_utils import run_bass_kernel_spmd

    x = np.asarray(x).astype(np.int64)
    ei = np.asarray(edge_index).astype(np.int64)
    emb = np.asarray(emb, np.float32)
    Wl1 = np.asarray(Wl1, np.float32); Wr1 = np.asarray(Wr1, np.float32)
    Wl2 = np.asarray(Wl2, np.float32); Wr2 = np.asarray(Wr2, np.float32)
    Wout = np.asarray(Wout, np.float32)
    bl1 = np.asarray(bl1, np.float32); bl2 = np.asarray(bl2, np.float32)
    bout = np.asarray(bout, np.float32)
    a1f = float(np.asarray(a1)); a2f = float(np.asarray(a2))
    src, dst = ei[0], ei[1]

    # ---- host prep ------------------------------------------------------
    # emb in transposed block layout [8*64, 12544]
    embT = np.zeros((NBLK * HID, RPC), np.float32)
    for b in range(NBLK):
        lo = b * RPC
        hi = min((b + 1) * RPC, emb.shape[0])
        embT[b * HID:(b + 1) * HID, :hi - lo] = emb[lo:hi].T

    # per-core h0_own^T and 1/cnt (replicated over 64 partitions)
    h0ownT = np.zeros((P, HID, RPC), np.float32)
    invcr = np.zeros((P, HID, RPC), np.float32)
    cnt = np.bincount(dst, minlength=N).astype(np.float32)
    for c in range(P):
        h0ownT[c, :, :NPC] = emb[x[c * NPC:(c + 1) * NPC]].T
        invcr[c, :, :NPC] = np.tile(
            (1.0 / np.maximum(cnt[c * NPC:(c + 1) * NPC], 1.0))[None, :],
            (HID, 1))

    calls1, gi1c, ui1c = _plan_layer(x[src], dst, RPC)
    calls2, gi2c, ui2c = _plan_layer(src, dst, NPC)
    tot1 = sum(cl["L"] for cl in calls1)
    tot2 = sum(cl["L"] for cl in calls2)
    print(f"plan: L1 gather idx/core {tot1} (pad {tot1 / (E / P):.3f}), "
          f"L2 {tot2} (pad {tot2 / (E / P):.3f})")

    # ---- device program -------------------------------------------------
    f32, i16 = mybir.dt.float32, mybir.dt.int16
    AX = mybir.AxisListType.X
    ADD = mybir.AluOpType.add
    MULT = mybir.AluOpType.mult
    PRELU = mybir.ActivationFunctionType.Prelu

    nc = bacc.Bacc()
    dp = nc.declare_dram_parameter
    embT_p = dp("embT", [NBLK * HID, RPC], f32, isOutput=False)
    h0o_p = dp("h0ownT", [HID, RPC], f32, isOutput=False)
    invc_p = dp("invcr", [HID, RPC], f32, isOutput=False)
    gi1_p = dp("gi1", list(gi1c[0].shape), i16, isOutput=False)
    gi2_p = dp("gi2", list(gi2c[0].shape), i16, isOutput=False)
    ui1_p = dp("ui1", list(ui1c[0].shape), i16, isOutput=False)
    ui2_p = dp("ui2", list(ui2c[0].shape), i16, isOutput=False)
    wl1_p = dp("wl1", [2 * HID, HID], f32, isOutput=False)
    wr1_p = dp("wr1", [HID, HID], f32, isOutput=False)
    wl2_p = dp("wl2", [2 * HID, HID], f32, isOutput=False)
    wr2_p = dp("wr2", [HID, HID], f32, isOutput=False)
    wout_p = dp("wout", [HID, HID], f32, isOutput=False)
    bl1_p = dp("bl1t", [HID, 1], f32, isOutput=False)
    bl2_p = dp("bl2t", [HID, 1], f32, isOutput=False)
    bout_p = dp("boutt", [HID, 1], f32, isOutput=False)
    out_p = dp("out", [HID, RPC], f32, isOutput=True)

    h1T_d = nc.dram_tensor("h1T", [HID, RPC], f32)
    hcatT_d = nc.dram_tensor("hcatT", [P * HID, RPC], f32, addr_space="Shared")

    GIMAX = max(cl["L"] // 16 for cl in calls1 + calls2)
    NUCH = [UCH] * (RPC // UCH) + ([RPC % UCH] if RPC % UCH else [])
    NPCH = [PCH] * (RPC // PCH) + ([RPC % PCH] if RPC % PCH else [])

    with tile.TileContext(nc) as tc:
        with tc.tile_pool(name="const", bufs=1) as cpool, \
             tc.tile_pool(name="big", bufs=1) as bpool, \
             tc.tile_pool(name="gio", bufs=2) as gpool, \
             tc.tile_pool(name="ph2", bufs=2) as qpool, \
             tc.tile_pool(name="ps", bufs=2, space="PSUM") as ppool:

            wl1_t = cpool.tile([2 * HID, HID], f32); nc.sync.dma_start(wl1_t[:], wl1_p[:])
            wr1_t = cpool.tile([HID, HID], f32); nc.sync.dma_start(wr1_t[:], wr1_p[:])
            wl2_t = cpool.tile([2 * HID, HID], f32); nc.sync.dma_start(wl2_t[:], wl2_p[:])
            wr2_t = cpool.tile([HID, HID], f32); nc.sync.dma_start(wr2_t[:], wr2_p[:])
            wout_t = cpool.tile([HID, HID], f32); nc.sync.dma_start(wout_t[:], wout_p[:])
            bl1_t = cpool.tile([HID, 1], f32); nc.sync.dma_start(bl1_t[:], bl1_p[:])
            bl2_t = cpool.tile([HID, 1], f32); nc.sync.dma_start(bl2_t[:], bl2_p[:])
            bout_t = cpool.tile([HID, 1], f32); nc.sync.dma_start(bout_t[:], bout_p[:])

            win = bpool.tile([128, WCOLS], f32)
            plane = bpool.tile([128, RPC], f32)
            acc = bpool.tile([128, RPC], f32)
            nc.vector.memset(win[:, RPC:WCOLS], 0.0)

            import os
            _bisect = os.environ.get("BISECT", "")
            _layers = (1,) if _bisect in ("l1", "l1c") else (1, 2)
            for layer in _layers:
                calls = calls1 if layer == 1 else calls2
                gi_p = gi1_p if layer == 1 else gi2_p
                ui_p = ui1_p if layer == 1 else ui2_p
                src_d = embT_p if layer == 1 else hcatT_d
                wl_t = wl1_t if layer == 1 else wl2_t
                wr_t = wr1_t if layer == 1 else wr2_t
                bl_t = bl1_t if layer == 1 else bl2_t
                hown_d = h0o_p if layer == 1 else h1T_d
                alpha = a1f if layer == 1 else a2f

                # ---- phase 1: gather + segment reduce + unpermute-add ----
                gofs = 0
                for j, cl in enumerate(calls):
                    nc.sync.dma_start(win[0:64, 0:RPC],
                                      src_d[(2 * j) * HID:(2 * j + 1) * HID])
                    nc.sync.dma_start(win[64:128, 0:RPC],
                                      src_d[(2 * j + 1) * HID:(2 * j + 2) * HID])
                    Lc = cl["L"] // 16
                    gi_t = gpool.tile([128, GIMAX], i16, tag="gi")
                    nc.sync.dma_start(gi_t[:, :Lc], gi_p[:, gofs:gofs + Lc])
                    gofs += Lc
                    nc.vector.memset(plane[0:64, cl["fzA"]:RPC], 0.0)
                    nc.vector.memset(plane[64:128, cl["fzB"]:RPC], 0.0)
                    for k, cs in enumerate(cl["sizes"]):
                        gout = gpool.tile([128, CMAX], f32, tag="g")
                        nc.gpsimd.ap_gather(
                            gout[:, :cs], win[:], gi_t[:, k * (CMAX // 16):k * (CMAX // 16) + cs // 16],
                            128, WCOLS, 1, cs)
                        for h in (0, 1):
                            p0 = 64 * h
                            for (loff, pos, T, D) in cl["chunk_tiles"][k][h]:
                                view = gout[p0:p0 + 64, loff:loff + T * D] \
                                    .rearrange("p (t d) -> p t d", d=D)
                                nc.vector.tensor_reduce(
                                    plane[p0:p0 + 64, pos:pos + T], view,
                                    axis=AX, op=ADD)
                    ui_t = gpool.tile([128, RPC // 16], i16, tag="ui")
                    nc.sync.dma_start(ui_t[:],
                                      ui_p[:, j * (RPC // 16):(j + 1) * (RPC // 16)])
                    n0 = 0
                    for cs in NUCH:
                        ut = gpool.tile([128, UCH], f32, tag="u")
                        nc.gpsimd.ap_gather(
                            ut[:, :cs], plane[:],
                            ui_t[:, n0 // 16:(n0 + cs) // 16],
                            128, RPC, 1, cs)
                        if j == 0:
                            nc.vector.tensor_copy(acc[:, n0:n0 + cs], ut[:, :cs])
                        else:
                            nc.vector.tensor_tensor(
                                acc[:, n0:n0 + cs], acc[:, n0:n0 + cs],
                                ut[:, :cs], op=ADD)
                        n0 += cs

                if _bisect == "p1":
                    nc.sync.dma_start(out_p[:, 0:RPC], acc[0:64, :])
                    break
                # ---- phase 2: mean-scale + MLP ----
                n0 = 0
                for cs in NPCH:
                    invcc = qpool.tile([HID, PCH], f32, tag="ic")
                    nc.sync.dma_start(invcc[:, :cs], invc_p[:, n0:n0 + cs])
                    hoc = qpool.tile([HID, PCH], f32, tag="ho")
                    nc.sync.dma_start(hoc[:, :cs], hown_d[:, n0:n0 + cs])
                    psA = ppool.tile([HID, PCH], f32, tag="pa")
                    nc.tensor.matmul(psA[:, :cs], wl_t[0:64, :], acc[0:64, n0:n0 + cs],
                                     start=True, stop=True)
                    psC = ppool.tile([HID, PCH], f32, tag="pc")
                    nc.tensor.matmul(psC[:, :cs], wl_t[64:128, :], acc[64:128, n0:n0 + cs],
                                     start=True, stop=True)
                    if _bisect == "p2m":
                        ocx = qpool.tile([HID, PCH], f32, tag="oc")
                        nc.vector.tensor_copy(ocx[:, :cs], psA[:, :cs])
                        nc.sync.dma_start(out_p[:, n0:n0 + cs], ocx[:, :cs])
                        n0 += cs
                        continue
                    psB = ppool.tile([HID, PCH], f32, tag="pb")
                    nc.tensor.matmul(psB[:, :cs], wr_t[:], hoc[:, :cs],
                                     start=True, stop=True)
                    t1 = qpool.tile([HID, PCH], f32, tag="t1")
                    nc.vector.tensor_tensor(t1[:, :cs], psA[:, :cs],
                                            invcc[:, :cs], op=MULT)
                    t1c = qpool.tile([HID, PCH], f32, tag="t1c")
                    nc.vector.tensor_tensor(t1c[:, :cs], psC[:, :cs],
                                            invcc[:, :cs], op=MULT)
                    u1 = qpool.tile([HID, PCH], f32, tag="u1")
                    nc.vector.tensor_tensor(u1[:, :cs], t1[:, :cs],
                                            t1c[:, :cs], op=ADD)
                    t2 = ppool.tile([HID, PCH], f32, tag="pt")
                    nc.vector.tensor_tensor(t2[:, :cs], u1[:, :cs],
                                            psB[:, :cs], op=ADD)
                    hout = qpool.tile([HID, PCH], f32, tag="hh")
                    nc.scalar.activation(hout[:, :cs], t2[:, :cs], PRELU,
                                         bias=bl_t[:], alpha=alpha)
                    if layer == 1:
                        nc.sync.dma_start(h1T_d[:, n0:n0 + cs], hout[:, :cs])
                        if _bisect in ("l1", "l1c"):
                            nc.sync.dma_start(out_p[:, n0:n0 + cs], hout[:, :cs])
                    else:
                        psO = ppool.tile([HID, PCH], f32, tag="pa")
                        nc.tensor.matmul(psO[:, :cs], wout_t[:], hout[:, :cs],
                                         start=True, stop=True)
                        oc = qpool.tile([HID, PCH], f32, tag="oc")
                        nc.vector.tensor_scalar_add(oc[:, :cs], psO[:, :cs],
                                                    bout_t[:])
                        nc.sync.dma_start(out_p[:, n0:n0 + cs], oc[:, :cs])
                    n0 += cs

                if _bisect == "p2m":
                    break
                if layer == 1 and _bisect != "l1":
                    nc.gpsimd.collective_compute(
                        "AllGather", mybir.AluOpType.bypass,
                        replica_groups=[list(range(P))],
                        ins=[h1T_d[:]], outs=[hcatT_d[:]])

    nc.compile()

    in_maps = []
    for c in range(P):
        in_maps.append({
            "embT": embT, "h0ownT": h0ownT[c], "invcr": invcr[c],
            "gi1": gi1c[c], "gi2": gi2c[c], "ui1": ui1c[c], "ui2": ui2c[c],
            "wl1": np.concatenate([Wl1, Wl1]), "wr1": Wr1,
            "wl2": np.concatenate([Wl2, Wl2]), "wr2": Wr2, "wout": Wout,
            "bl1t": bl1.reshape(HID, 1), "bl2t": bl2.reshape(HID, 1),
            "boutt": bout.reshape(HID, 1),
        })
    res = run_bass_kernel_spmd(nc, in_maps, list(range(P)))
    out = np.zeros((N, HID), np.float32)
    for c in range(P):
        out[c * NPC:(c + 1) * NPC] = res.results[c]["out"][:, :NPC].T
    kernel.last_exec_time_ns = res.exec_time_ns
    return out


# revision 16
# speedup vs baseline: 2.9267x; 2.9267x over previous
"""GNN (2x SAGEConv + linear) Bass kernel for trn2, 8 NeuronCores.

Sharding: nodes partitioned across 8 cores (12500 each, dst-range).
Each layer: per-core windowed padded-CSR gathers of h[src] (dma_gather,
int16 windows of 25088 hcat rows), on-chip segment reduce (DVE strided),
batched unique-row dma_scatter_add into per-window DRAM accumulators,
dense combine + PE MLP.  One AllGather of h1 slices between layers.
"""
import numpy as np

N = 100000
E = 1250000
HID = 64
P = 8
NPC = 12500          # nodes per core
RPC = 12544          # rows per core block (98 * 128), rows 12500+ are zero pads
NB = RPC // 128      # 98 blocks
WIN = 2 * RPC        # 25088 rows per gather window (2 rank blocks)
NW = 4               # windows
ZLOC = 12500         # local row inside a window that is guaranteed zero
ACCR = RPC + 128     # accumulator rows (tail rows are scratch)
MAXPOS = 4096        # max gather positions per call
MAXT = 16            # max tiles per gather call


def _hcat_local(sid):
    """window and local index of global node/emb row sid in hcat layout."""
    w = sid // (2 * NPC)
    loc = RPC * ((sid // NPC) % 2) + sid % NPC
    return w, loc


def _wrap128(vals):
    """flat int16 stream -> [128, len/16] wrapped+replicated layout."""
    n = vals.shape[0]
    w16 = np.ascontiguousarray(vals.reshape(n // 16, 16).T)
    return np.tile(w16, (8, 1))


def _build_layer_meta(sid, dst, rng_pad):
    """Per-layer gather/scatter metadata.

    sid: effective source row (hcat-layout global id source) per edge [E]
    dst: destination node per edge [E]
    Returns: per-core index arrays + shared compile-time group structure.
    """
    core = dst // NPC
    dstl = dst % NPC
    w_of = sid // (2 * NPC)
    loc = RPC * ((sid // NPC) % 2) + sid % NPC

    # adjacency per (core, window): lists of local src
    # order: per (core,window) per dst node
    deg = np.zeros((P, NW, RPC), np.int32)
    np.add.at(deg, (core, w_of, dstl), 1)

    # sorted node order per (core, window): by degree desc; all RPC slots
    order = np.argsort(-deg, axis=2, kind="stable")  # [P, NW, RPC]

    # tile max-degrees unified over cores
    deg_sorted = -np.sort(-deg, axis=2)              # [P, NW, RPC]
    tile_max = deg_sorted.reshape(P, NW, NB, 128).max(axis=3)  # [P,NW,NB]
    D = tile_max.max(axis=0)                         # [NW, NB] shared

    # group structure (shared): per window pack tiles into calls
    groups = []  # list over windows of list of (tile_idx list, D list)
    for w in range(NW):
        gw = []
        cur, curpos = [], 0
        for t in range(NB):
            d = int(D[w, t])
            if d == 0:
                continue
            if cur and (curpos + d * 128 > MAXPOS or len(cur) >= MAXT):
                gw.append(cur)
                cur, curpos = [], 0
            cur.append(t)
            curpos += d * 128
        if cur:
            gw.append(cur)
        groups.append(gw)

    # per-core flat gather idx + scatter idx streams
    # edge lists per (core, window, dstl) in input order
    eorder = np.lexsort((loc, dstl, w_of, core))
    sc, sw, sd, sl = core[eorder], w_of[eorder], dstl[eorder], loc[eorder]
    # starts of each (core, window, node) run
    key = ((sc * NW + sw) * RPC + sd).astype(np.int64)
    starts = np.searchsorted(key, np.arange(P * NW * RPC, dtype=np.int64))
    starts = np.append(starts, len(key))

    gidx_cores, sidx_cores = [], []
    for k in range(P):
        gparts, sparts = [], []
        for w in range(NW):
            od = order[k, w]
            for gt in groups[w]:
                for t in gt:
                    d = int(D[w, t])
                    nodes = od[t * 128:(t + 1) * 128]
                    blockg = np.full((d, 128), ZLOC, np.int32)
                    for p in range(128):
                        nloc = int(nodes[p])
                        s0 = starts[(k * NW + w) * RPC + nloc]
                        s1 = starts[(k * NW + w) * RPC + nloc + 1]
                        cnt = s1 - s0
                        if cnt:
                            blockg[:cnt, p] = sl[s0:s1]
                    gparts.append(blockg.reshape(-1))
                # scatter rows for this group: tiles' node ids
                srows = od[np.array(gt) ]  # placeholder replaced below
                srows = np.concatenate(
                    [od[t * 128:(t + 1) * 128] for t in gt]).astype(np.int32)
                sparts.append(srows)
        gidx_cores.append(_wrap128(np.concatenate(gparts).astype(np.int16)))
        sidx_cores.append(_wrap128(np.concatenate(sparts).astype(np.int16)))
    return groups, D, gidx_cores, sidx_cores


def kernel(x, edge_index, edge_weight, emb, Wl1, bl1, Wr1, a1,
           Wl2, bl2, Wr2, a2, Wout, bout):
    import concourse.bacc as bacc
    import concourse.mybir as mybir
    import concourse.tile as tile
    from concourse.bass_utils import run_bass_kernel_spmd
    from concourse.masks import make_identity

    x = np.asarray(x).astype(np.int64)
    ei = np.asarray(edge_index).astype(np.int64)
    emb = np.asarray(emb, np.float32)
    Wl1 = np.asarray(Wl1, np.float32); Wr1 = np.asarray(Wr1, np.float32)
    Wl2 = np.asarray(Wl2, np.float32); Wr2 = np.asarray(Wr2, np.float32)
    Wout = np.asarray(Wout, np.float32)
    bl1 = np.asarray(bl1, np.float32); bl2 = np.asarray(bl2, np.float32)
    bout = np.asarray(bout, np.float32)
    a1f = float(np.asarray(a1)); a2f = float(np.asarray(a2))
    src, dst = ei[0], ei[1]

    # ---- host prep ------------------------------------------------------
    # emb in hcat layout [8*RPC, HID]
    emb_hc = np.zeros((P * RPC, HID), np.float32)
    for r in range(P):
        emb_hc[r * RPC:r * RPC + NPC] = emb[r * NPC:(r + 1) * NPC]

    # per-core own h0 = emb[x[own]]
    h0_own = np.zeros((P, RPC, HID), np.float32)
    for k in range(P):
        h0_own[k, :NPC] = emb[x[k * NPC:(k + 1) * NPC]]

    # inverse counts (node order, [128, NB] partition-major)
    cnt = np.bincount(dst, minlength=N).astype(np.float32)
    invc = np.zeros((P, 128, NB), np.float32)
    for k in range(P):
        c = np.zeros(RPC, np.float32)
        c[:NPC] = 1.0 / np.maximum(cnt[k * NPC:(k + 1) * NPC], 1.0)
        invc[k] = c.reshape(NB, 128).T

    g1, D1, gidx1, sidx1 = _build_layer_meta(x[src], dst, 0)
    g2, D2, gidx2, sidx2 = _build_layer_meta(src, dst, 0)

    # ---- device program -------------------------------------------------
    f32, i16 = mybir.dt.float32, mybir.dt.int16
    nc = bacc.Bacc(dynamic_dma_scratch_size=65536, num_swdge_queues=4)
    dp = nc.declare_dram_parameter
    embw = dp("embw", [P * RPC, HID], f32, isOutput=False)
    h0o = dp("h0o", [RPC, HID], f32, isOutput=False)
    gi1 = dp("gi1", list(gidx1[0].shape), i16, isOutput=False)
    si1 = dp("si1", list(sidx1[0].shape), i16, isOutput=False)
    gi2 = dp("gi2", list(gidx2[0].shape), i16, isOutput=False)
    si2 = dp("si2", list(sidx2[0].shape), i16, isOutput=False)
    invc_p = dp("invc", [128, NB], f32, isOutput=False)
    wl1_p = dp("wl1", [HID, HID], f32, isOutput=False)
    wr1_p = dp("wr1", [HID, HID], f32, isOutput=False)
    wl2_p = dp("wl2", [HID, HID], f32, isOutput=False)
    wr2_p = dp("wr2", [HID, HID], f32, isOutput=False)
    wout_p = dp("wout", [HID, HID], f32, isOutput=False)
    bl1_p = dp("bl1t", [HID, 1], f32, isOutput=False)
    bl2_p = dp("bl2t", [HID, 1], f32, isOutput=False)
    bout_p = dp("boutr", [128, HID], f32, isOutput=False)
    out_p = dp("out", [RPC, HID], f32, isOutput=True)

    acc_d = [nc.dram_tensor(f"acc{w}", [ACCR, HID], f32) for w in range(NW)]
    hc1_d = nc.dram_tensor("hc1", [RPC, HID], f32)
    hcat1_d = nc.dram_tensor("hcat1", [P * RPC, HID], f32, addr_space="Shared")

    AX = mybir.AxisListType.X
    ADD = mybir.AluOpType.add
    PRELU = mybir.ActivationFunctionType.Prelu

    with tile.TileContext(nc) as tc:
        with tc.tile_pool(name="const", bufs=1) as cpool, \
             tc.tile_pool(name="big", bufs=1) as bpool, \
             tc.tile_pool(name="gio", bufs=2) as gpool, \
             tc.tile_pool(name="ph2", bufs=3) as qpool, \
             tc.tile_pool(name="ps", bufs=1, space="PSUM") as ppool:

            ident = cpool.tile([128, 128], f32)
            make_identity(nc, ident[:])
            wl1_t = cpool.tile([HID, HID], f32); nc.sync.dma_start(wl1_t[:], wl1_p[:])
            wr1_t = cpool.tile([HID, HID], f32); nc.sync.dma_start(wr1_t[:], wr1_p[:])
            wl2_t = cpool.tile([HID, HID], f32); nc.sync.dma_start(wl2_t[:], wl2_p[:])
            wr2_t = cpool.tile([HID, HID], f32); nc.sync.dma_start(wr2_t[:], wr2_p[:])
            wout_t = cpool.tile([HID, HID], f32); nc.sync.dma_start(wout_t[:], wout_p[:])
            bl1_t = cpool.tile([HID, 1], f32); nc.sync.dma_start(bl1_t[:], bl1_p[:])
            bl2_t = cpool.tile([HID, 1], f32); nc.sync.dma_start(bl2_t[:], bl2_p[:])
            bout_t = cpool.tile([128, HID], f32); nc.sync.dma_start(bout_t[:], bout_p[:])
            invc_t = cpool.tile([128, NB], f32); nc.sync.dma_start(invc_t[:], invc_p[:])

            h1T = bpool.tile([HID, NB, 128], f32)      # h1 transposed, own nodes
            hc1_t = bpool.tile([128, NB, HID], f32)    # h1 node-major, own nodes
            zt = cpool.tile([128, HID], f32)
            nc.vector.memset(zt[:], 0.0)
            zbig = cpool.tile([128, 33, HID], f32)
            nc.vector.memset(zbig[:], 0.0)

            def zero_accs():
                for w in range(NW):
                    dstv = acc_d[w][:].rearrange("(r p) f -> p r f", p=128)
                    for c in range(3):
                        nc.sync.dma_start(dstv[:, c * 33:(c + 1) * 33, :], zbig[:])

            def phase1(groups, D, gi_p, si_p, src_dram):
                gi_t = bpool.tile([128, gi_p.shape[1]], i16, tag="gi")
                si_t = bpool.tile([128, si_p.shape[1]], i16, tag="si")
                nc.sync.dma_start(gi_t[:], gi_p[:])
                nc.sync.dma_start(si_t[:], si_p[:])
                gcol = 0
                scol = 0
                qn = 0
                for w in range(NW):
                    win = src_dram[w * WIN:(w + 1) * WIN]
                    for gt in groups[w]:
                        npos = int(sum(D[w, t] for t in gt)) * 128
                        ncols = npos // 128
                        nt = len(gt)
                        g_t = gpool.tile([128, MAXPOS // 128, HID], f32, tag="g")
                        r_t = gpool.tile([128, MAXT, HID], f32, tag="r")
                        nc.gpsimd.dma_gather(
                            g_t[:, :ncols, :], win, gi_t[:, gcol:gcol + npos // 16],
                            npos, npos, HID, single_packet=False,
                            queue_num=qn % 3)
                        off = 0
                        for i, t in enumerate(gt):
                            d = int(D[w, t])
                            view = g_t[:, off:off + d, :].rearrange("p d f -> p f d")
                            nc.vector.tensor_reduce(r_t[:, i, :], view, axis=AX, op=ADD)
                            off += d
                        nc.gpsimd.dma_scatter_add(
                            acc_d[w][:], r_t[:, :nt, :], si_t[:, scol:scol + nt * 8],
                            nt * 128, nt * 128, HID, single_packet=False,
                            queue_num=3)
                        gcol += npos // 16
                        scol += nt * 8
                        qn += 1

            def phase2(L):
                wl_t = wl1_t if L == 1 else wl2_t
                wr_t = wr1_t if L == 1 else wr2_t
                bl_t = bl1_t if L == 1 else bl2_t
                alpha = a1f if L == 1 else a2f
                for b in range(NB):
                    m_t = qpool.tile([128, NW, HID], f32, tag="m")
                    for w in range(NW):
                        nc.sync.dma_start(m_t[:, w, :],
                                          acc_d[w][b * 128:(b + 1) * 128])
                    mean0 = qpool.tile([128, HID], f32, tag="mean0")
                    nc.vector.tensor_reduce(
                        mean0[:], m_t[:].rearrange("p w f -> p f w"), axis=AX, op=ADD)
                    meansc = qpool.tile([128, HID], f32, tag="meansc")
                    nc.vector.tensor_scalar_mul(meansc[:], mean0[:], invc_t[:, b:b + 1])
                    psA = ppool.tile([HID, 128], f32, tag="psA")
                    nc.tensor.transpose(psA[:], meansc[:], ident[:])
                    meanT = qpool.tile([HID, 128], f32, tag="meanT")
                    nc.vector.tensor_copy(meanT[:], psA[:])
                    if L == 1:
                        hob = qpool.tile([128, HID], f32, tag="hob")
                        nc.sync.dma_start(hob[:], h0o[b * 128:(b + 1) * 128])
                        psB = ppool.tile([HID, 128], f32, tag="psB")
                        nc.tensor.transpose(psB[:], hob[:], ident[:])
                        hT = qpool.tile([HID, 128], f32, tag="hT")
                        nc.vector.tensor_copy(hT[:], psB[:])
                        hT_ap = hT[:]
                    else:
                        hT_ap = h1T[:, b, :]
                    psC = ppool.tile([HID, 128], f32, tag="psC")
                    nc.tensor.matmul(psC[:], wl_t[:], meanT[:], start=True, stop=False)
                    nc.tensor.matmul(psC[:], wr_t[:], hT_ap, start=False, stop=True)
                    if L == 1:
                        nc.scalar.activation(h1T[:, b, :], psC[:], PRELU,
                                             bias=bl_t[:], alpha=alpha)
                        psD = ppool.tile([128, HID], f32, tag="psD")
                        nc.tensor.transpose(psD[:], h1T[:, b, :], ident[:HID, :HID])
                        nc.vector.tensor_copy(hc1_t[:, b, :], psD[:])
                    else:
                        h2T = qpool.tile([HID, 128], f32, tag="h2T")
                        nc.scalar.activation(h2T[:], psC[:], PRELU,
                                             bias=bl_t[:], alpha=alpha)
                        psE = ppool.tile([128, HID], f32, tag="psE")
                        nc.tensor.matmul(psE[:], h2T[:], wout_t[:], start=True, stop=True)
                        outb = qpool.tile([128, HID], f32, tag="outb")
                        nc.vector.tensor_tensor(outb[:], psE[:], bout_t[:], op=ADD)
                        nc.sync.dma_start(out_p[b * 128:(b + 1) * 128], outb[:])

            # ---- layer 1 ----
            zero_accs()
            phase1(g1, D1, gi1, si1, embw)
            phase2(1)
            # zero pad rows of h1 (nodes 12500..12543) so gather pads stay 0
            nc.vector.memset(h1T[:, NB - 1, 84:128], 0.0)
            nc.sync.dma_start(hc1_d[:].rearrange("(b p) f -> p b f", p=128), hc1_t[:])
            nc.sync.dma_start(hc1_d[NPC:RPC], zt[:RPC - NPC, :])
            nc.gpsimd.collective_compute(
                "AllGather", mybir.AluOpType.bypass,
                replica_groups=[list(range(P))],
                ins=[hc1_d[:]], outs=[hcat1_d[:]])
            # ---- layer 2 + out ----
            zero_accs()
            phase1(g2, D2, gi2, si2, hcat1_d)
            phase2(2)

    nc.compile()

    in_maps = []
    for k in range(P):
        in_maps.append({
            "embw": emb_hc, "h0o": h0_own[k],
            "gi1": gidx1[k], "si1": sidx1[k],
            "gi2": gidx2[k], "si2": sidx2[k],
            "invc": invc[k],
            "wl1": Wl1, "wr1": Wr1, "wl2": Wl2, "wr2": Wr2, "wout": Wout,
            "bl1t": bl1.reshape(HID, 1), "bl2t": bl2.reshape(HID, 1),
            "boutr": np.tile(bout.reshape(1, HID), (128, 1)),
        })
    res = run_bass_kernel_spmd(nc, in_maps, list(range(P)))
    out = np.zeros((N, HID), np.float32)
    for k in range(P):
        out[k * NPC:(k + 1) * NPC] = res.results[k]["out"][:NPC]
    kernel.last_exec_time_ns = res.exec_time_ns
    return out



# revision 17
# speedup vs baseline: 3.3882x; 1.1577x over previous
"""GNN (2x SAGEConv + linear) Bass kernel for trn2, 8 NeuronCores.

Sharding: nodes partitioned across 8 cores (12500 each, dst-range).
Each layer: per-core windowed padded-CSR gathers of h[src] (dma_gather,
int16 windows of 25088 hcat rows), on-chip segment reduce (DVE strided),
batched unique-row dma_scatter_add into per-window DRAM accumulators,
dense combine + PE MLP.  One AllGather of h1 slices between layers.
"""
import numpy as np

N = 100000
E = 1250000
HID = 64
P = 8
NPC = 12500          # nodes per core
RPC = 12544          # rows per core block (98 * 128), rows 12500+ are zero pads
NB = RPC // 128      # 98 blocks
WIN = 2 * RPC        # 25088 rows per gather window (2 rank blocks)
NW = 4               # windows
ZLOC = 12500         # local row inside a window that is guaranteed zero
ACCR = RPC + 128     # accumulator rows (tail rows are scratch)
MAXPOS = 2048        # max gather positions per call
MAXT = 16            # max tiles per gather call


def _hcat_local(sid):
    """window and local index of global node/emb row sid in hcat layout."""
    w = sid // (2 * NPC)
    loc = RPC * ((sid // NPC) % 2) + sid % NPC
    return w, loc


def _wrap128(vals):
    """flat int16 stream -> [128, len/16] wrapped+replicated layout."""
    n = vals.shape[0]
    w16 = np.ascontiguousarray(vals.reshape(n // 16, 16).T)
    return np.tile(w16, (8, 1))


def _build_layer_meta(sid, dst, rng_pad):
    """Per-layer gather/scatter metadata.

    sid: effective source row (hcat-layout global id source) per edge [E]
    dst: destination node per edge [E]
    Returns: per-core index arrays + shared compile-time group structure.
    """
    core = dst // NPC
    dstl = dst % NPC
    w_of = sid // (2 * NPC)
    loc = RPC * ((sid // NPC) % 2) + sid % NPC

    # adjacency per (core, window): lists of local src
    # order: per (core,window) per dst node
    deg = np.zeros((P, NW, RPC), np.int32)
    np.add.at(deg, (core, w_of, dstl), 1)

    # sorted node order per (core, window): by degree desc; all RPC slots
    order = np.argsort(-deg, axis=2, kind="stable")  # [P, NW, RPC]

    # tile max-degrees unified over cores
    deg_sorted = -np.sort(-deg, axis=2)              # [P, NW, RPC]
    tile_max = deg_sorted.reshape(P, NW, NB, 128).max(axis=3)  # [P,NW,NB]
    D = tile_max.max(axis=0)                         # [NW, NB] shared

    # group structure (shared): per window pack tiles into calls
    groups = []  # list over windows of list of (tile_idx list, D list)
    for w in range(NW):
        gw = []
        cur, curpos = [], 0
        for t in range(NB):
            d = int(D[w, t])
            if d == 0:
                continue
            if cur and (curpos + d * 128 > MAXPOS or len(cur) >= MAXT):
                gw.append(cur)
                cur, curpos = [], 0
            cur.append(t)
            curpos += d * 128
        if cur:
            gw.append(cur)
        groups.append(gw)

    # per-core flat gather idx + scatter idx streams
    # edge lists per (core, window, dstl) in input order
    eorder = np.lexsort((loc, dstl, w_of, core))
    sc, sw, sd, sl = core[eorder], w_of[eorder], dstl[eorder], loc[eorder]
    # starts of each (core, window, node) run
    key = ((sc * NW + sw) * RPC + sd).astype(np.int64)
    starts = np.searchsorted(key, np.arange(P * NW * RPC, dtype=np.int64))
    starts = np.append(starts, len(key))

    gidx_cores, sidx_cores = [], []
    for k in range(P):
        gparts, sparts = [], []
        for w in range(NW):
            od = order[k, w]
            for gt in groups[w]:
                for t in gt:
                    d = int(D[w, t])
                    nodes = od[t * 128:(t + 1) * 128]
                    blockg = np.full((d, 128), ZLOC, np.int32)
                    for p in range(128):
                        nloc = int(nodes[p])
                        s0 = starts[(k * NW + w) * RPC + nloc]
                        s1 = starts[(k * NW + w) * RPC + nloc + 1]
                        cnt = s1 - s0
                        if cnt:
                            blockg[:cnt, p] = sl[s0:s1]
                    gparts.append(blockg.reshape(-1))
                # scatter rows for this group: tiles' node ids
                srows = od[np.array(gt) ]  # placeholder replaced below
                srows = np.concatenate(
                    [od[t * 128:(t + 1) * 128] for t in gt]).astype(np.int32)
                sparts.append(srows)
        gidx_cores.append(_wrap128(np.concatenate(gparts).astype(np.int16)))
        sidx_cores.append(_wrap128(np.concatenate(sparts).astype(np.int16)))
    return groups, D, gidx_cores, sidx_cores


def kernel(x, edge_index, edge_weight, emb, Wl1, bl1, Wr1, a1,
           Wl2, bl2, Wr2, a2, Wout, bout):
    import concourse.bacc as bacc
    import concourse.mybir as mybir
    import concourse.tile as tile
    from concourse.bass_utils import run_bass_kernel_spmd
    from concourse.masks import make_identity

    x = np.asarray(x).astype(np.int64)
    ei = np.asarray(edge_index).astype(np.int64)
    emb = np.asarray(emb, np.float32)
    Wl1 = np.asarray(Wl1, np.float32); Wr1 = np.asarray(Wr1, np.float32)
    Wl2 = np.asarray(Wl2, np.float32); Wr2 = np.asarray(Wr2, np.float32)
    Wout = np.asarray(Wout, np.float32)
    bl1 = np.asarray(bl1, np.float32); bl2 = np.asarray(bl2, np.float32)
    bout = np.asarray(bout, np.float32)
    a1f = float(np.asarray(a1)); a2f = float(np.asarray(a2))
    src, dst = ei[0], ei[1]

    # ---- host prep ------------------------------------------------------
    # emb in hcat layout [8*RPC, HID]
    emb_hc = np.zeros((P * RPC, HID), np.float32)
    for r in range(P):
        emb_hc[r * RPC:r * RPC + NPC] = emb[r * NPC:(r + 1) * NPC]

    # per-core own h0 = emb[x[own]]
    h0_own = np.zeros((P, RPC, HID), np.float32)
    for k in range(P):
        h0_own[k, :NPC] = emb[x[k * NPC:(k + 1) * NPC]]

    # inverse counts (node order, [128, NB] partition-major)
    cnt = np.bincount(dst, minlength=N).astype(np.float32)
    invc = np.zeros((P, 128, NB), np.float32)
    for k in range(P):
        c = np.zeros(RPC, np.float32)
        c[:NPC] = 1.0 / np.maximum(cnt[k * NPC:(k + 1) * NPC], 1.0)
        invc[k] = c.reshape(NB, 128).T

    g1, D1, gidx1, sidx1 = _build_layer_meta(x[src], dst, 0)
    g2, D2, gidx2, sidx2 = _build_layer_meta(src, dst, 0)

    # ---- device program -------------------------------------------------
    f32, i16 = mybir.dt.float32, mybir.dt.int16
    nc = bacc.Bacc(dynamic_dma_scratch_size=65536, num_swdge_queues=4)
    dp = nc.declare_dram_parameter
    embw = dp("embw", [P * RPC, HID], f32, isOutput=False)
    h0o = dp("h0o", [RPC, HID], f32, isOutput=False)
    gi1 = dp("gi1", list(gidx1[0].shape), i16, isOutput=False)
    si1 = dp("si1", list(sidx1[0].shape), i16, isOutput=False)
    gi2 = dp("gi2", list(gidx2[0].shape), i16, isOutput=False)
    si2 = dp("si2", list(sidx2[0].shape), i16, isOutput=False)
    invc_p = dp("invc", [128, NB], f32, isOutput=False)
    wl1_p = dp("wl1", [HID, HID], f32, isOutput=False)
    wr1_p = dp("wr1", [HID, HID], f32, isOutput=False)
    wl2_p = dp("wl2", [HID, HID], f32, isOutput=False)
    wr2_p = dp("wr2", [HID, HID], f32, isOutput=False)
    wout_p = dp("wout", [HID, HID], f32, isOutput=False)
    bl1_p = dp("bl1t", [HID, 1], f32, isOutput=False)
    bl2_p = dp("bl2t", [HID, 1], f32, isOutput=False)
    bout_p = dp("boutr", [128, HID], f32, isOutput=False)
    out_p = dp("out", [RPC, HID], f32, isOutput=True)

    acc_d = [nc.dram_tensor(f"acc{w}", [ACCR, HID], f32) for w in range(NW)]
    hc1_d = nc.dram_tensor("hc1", [RPC, HID], f32)
    hcat1_d = nc.dram_tensor("hcat1", [P * RPC, HID], f32, addr_space="Shared")

    AX = mybir.AxisListType.X
    ADD = mybir.AluOpType.add
    PRELU = mybir.ActivationFunctionType.Prelu

    with tile.TileContext(nc) as tc:
        with tc.tile_pool(name="const", bufs=1) as cpool, \
             tc.tile_pool(name="big", bufs=1) as bpool, \
             tc.tile_pool(name="gio", bufs=3) as gpool, \
             tc.tile_pool(name="ph2", bufs=3) as qpool, \
             tc.tile_pool(name="ps", bufs=1, space="PSUM") as ppool:

            ident = cpool.tile([128, 128], f32)
            make_identity(nc, ident[:])
            wl1_t = cpool.tile([HID, HID], f32); nc.sync.dma_start(wl1_t[:], wl1_p[:])
            wr1_t = cpool.tile([HID, HID], f32); nc.sync.dma_start(wr1_t[:], wr1_p[:])
            wl2_t = cpool.tile([HID, HID], f32); nc.sync.dma_start(wl2_t[:], wl2_p[:])
            wr2_t = cpool.tile([HID, HID], f32); nc.sync.dma_start(wr2_t[:], wr2_p[:])
            wout_t = cpool.tile([HID, HID], f32); nc.sync.dma_start(wout_t[:], wout_p[:])
            bl1_t = cpool.tile([HID, 1], f32); nc.sync.dma_start(bl1_t[:], bl1_p[:])
            bl2_t = cpool.tile([HID, 1], f32); nc.sync.dma_start(bl2_t[:], bl2_p[:])
            bout_t = cpool.tile([128, HID], f32); nc.sync.dma_start(bout_t[:], bout_p[:])
            invc_t = cpool.tile([128, NB], f32); nc.sync.dma_start(invc_t[:], invc_p[:])

            h1T = bpool.tile([HID, NB, 128], f32)      # h1 transposed, own nodes
            hc1_t = bpool.tile([128, NB, HID], f32)    # h1 node-major, own nodes
            zt = cpool.tile([128, HID], f32)
            nc.vector.memset(zt[:], 0.0)
            zbig = cpool.tile([128, 33, HID], f32)
            nc.vector.memset(zbig[:], 0.0)

            def zero_accs():
                for w in range(NW):
                    dstv = acc_d[w][:].rearrange("(r p) f -> p r f", p=128)
                    for c in range(3):
                        nc.sync.dma_start(dstv[:, c * 33:(c + 1) * 33, :], zbig[:])

            def phase1(groups, D, gi_p, si_p, src_dram):
                gi_t = bpool.tile([128, gi_p.shape[1]], i16, tag="gi")
                si_t = bpool.tile([128, si_p.shape[1]], i16, tag="si")
                nc.sync.dma_start(gi_t[:], gi_p[:])
                nc.sync.dma_start(si_t[:], si_p[:])
                gcol = 0
                scol = 0
                qn = 0
                for w in range(NW):
                    win = src_dram[w * WIN:(w + 1) * WIN]
                    for gt in groups[w]:
                        npos = int(sum(D[w, t] for t in gt)) * 128
                        ncols = npos // 128
                        nt = len(gt)
                        g_t = gpool.tile([128, MAXPOS // 128, HID], f32, tag="g")
                        r_t = gpool.tile([128, MAXT, HID], f32, tag="r")
                        nc.gpsimd.dma_gather(
                            g_t[:, :ncols, :], win, gi_t[:, gcol:gcol + npos // 16],
                            npos, npos, HID, single_packet=False,
                            queue_num=qn % 3)
                        off = 0
                        for i, t in enumerate(gt):
                            d = int(D[w, t])
                            view = g_t[:, off:off + d, :].rearrange("p d f -> p f d")
                            nc.vector.tensor_reduce(r_t[:, i, :], view, axis=AX, op=ADD)
                            off += d
                        nc.gpsimd.dma_scatter_add(
                            acc_d[w][:], r_t[:, :nt, :], si_t[:, scol:scol + nt * 8],
                            nt * 128, nt * 128, HID, single_packet=False,
                            queue_num=3)
                        gcol += npos // 16
                        scol += nt * 8
                        qn += 1

            def phase2(L):
                wl_t = wl1_t if L == 1 else wl2_t
                wr_t = wr1_t if L == 1 else wr2_t
                bl_t = bl1_t if L == 1 else bl2_t
                alpha = a1f if L == 1 else a2f
                for b in range(NB):
                    m_t = qpool.tile([128, NW, HID], f32, tag="m")
                    for w in range(NW):
                        nc.sync.dma_start(m_t[:, w, :],
                                          acc_d[w][b * 128:(b + 1) * 128])
                    mean0 = qpool.tile([128, HID], f32, tag="mean0")
                    nc.vector.tensor_reduce(
                        mean0[:], m_t[:].rearrange("p w f -> p f w"), axis=AX, op=ADD)
                    meansc = qpool.tile([128, HID], f32, tag="meansc")
                    nc.vector.tensor_scalar_mul(meansc[:], mean0[:], invc_t[:, b:b + 1])
                    psA = ppool.tile([HID, 128], f32, tag="psA")
                    nc.tensor.transpose(psA[:], meansc[:], ident[:])
                    meanT = qpool.tile([HID, 128], f32, tag="meanT")
                    nc.vector.tensor_copy(meanT[:], psA[:])
                    if L == 1:
                        hob = qpool.tile([128, HID], f32, tag="hob")
                        nc.sync.dma_start(hob[:], h0o[b * 128:(b + 1) * 128])
                        psB = ppool.tile([HID, 128], f32, tag="psB")
                        nc.tensor.transpose(psB[:], hob[:], ident[:])
                        hT = qpool.tile([HID, 128], f32, tag="hT")
                        nc.vector.tensor_copy(hT[:], psB[:])
                        hT_ap = hT[:]
                    else:
                        hT_ap = h1T[:, b, :]
                    psC = ppool.tile([HID, 128], f32, tag="psC")
                    nc.tensor.matmul(psC[:], wl_t[:], meanT[:], start=True, stop=False)
                    nc.tensor.matmul(psC[:], wr_t[:], hT_ap, start=False, stop=True)
                    if L == 1:
                        nc.scalar.activation(h1T[:, b, :], psC[:], PRELU,
                                             bias=bl_t[:], alpha=alpha)
                        psD = ppool.tile([128, HID], f32, tag="psD")
                        nc.tensor.transpose(psD[:], h1T[:, b, :], ident[:HID, :HID])
                        nc.vector.tensor_copy(hc1_t[:, b, :], psD[:])
                    else:
                        h2T = qpool.tile([HID, 128], f32, tag="h2T")
                        nc.scalar.activation(h2T[:], psC[:], PRELU,
                                             bias=bl_t[:], alpha=alpha)
                        psE = ppool.tile([128, HID], f32, tag="psE")
                        nc.tensor.matmul(psE[:], h2T[:], wout_t[:], start=True, stop=True)
                        outb = qpool.tile([128, HID], f32, tag="outb")
                        nc.vector.tensor_tensor(outb[:], psE[:], bout_t[:], op=ADD)
                        nc.sync.dma_start(out_p[b * 128:(b + 1) * 128], outb[:])

            # ---- layer 1 ----
            zero_accs()
            phase1(g1, D1, gi1, si1, embw)
            phase2(1)
            # zero pad rows of h1 (nodes 12500..12543) so gather pads stay 0
            nc.vector.memset(h1T[:, NB - 1, 84:128], 0.0)
            nc.sync.dma_start(hc1_d[:].rearrange("(b p) f -> p b f", p=128), hc1_t[:])
            nc.sync.dma_start(hc1_d[NPC:RPC], zt[:RPC - NPC, :])
            nc.gpsimd.collective_compute(
                "AllGather", mybir.AluOpType.bypass,
                replica_groups=[list(range(P))],
                ins=[hc1_d[:]], outs=[hcat1_d[:]])
            # ---- layer 2 + out ----
            zero_accs()
            phase1(g2, D2, gi2, si2, hcat1_d)
            phase2(2)

    nc.compile()

    in_maps = []
    for k in range(P):
        in_maps.append({
            "embw": emb_hc, "h0o": h0_own[k],
            "gi1": gidx1[k], "si1": sidx1[k],
            "gi2": gidx2[k], "si2": sidx2[k],
            "invc": invc[k],
            "wl1": Wl1, "wr1": Wr1, "wl2": Wl2, "wr2": Wr2, "wout": Wout,
            "bl1t": bl1.reshape(HID, 1), "bl2t": bl2.reshape(HID, 1),
            "boutr": np.tile(bout.reshape(1, HID), (128, 1)),
        })
    res = run_bass_kernel_spmd(nc, in_maps, list(range(P)))
    out = np.zeros((N, HID), np.float32)
    for k in range(P):
        out[k * NPC:(k + 1) * NPC] = res.results[k]["out"][:NPC]
    kernel.last_exec_time_ns = res.exec_time_ns
    return out

